# revision 36
# baseline (speedup 1.0000x reference)
"""DMGCN message-passing GNN on 8 Trainium2 NeuronCores (Bass/Tile).

Sharding: host load-balances nodes into 8*99 tiles of <=128 nodes and <=512
edges (snake-pack by degree), so every dst-tile is exactly tmax=4 128-edge
subtiles — uniform SPMD structure with ~1% padding. Per layer: f16 node-MLP
on own shard -> AllGather hn table -> z-chain (edge MLP; [emb|rbf] features
built once via one-hot matmuls + f32 RBF matmul, cached in DRAM) writing ee
into SBUF -> consume (indirect-gather hn[src], product, message matmul,
one-hot scatter matmuls emitting node-major PSUM directly) -> h update.
Readout on device, graph segment-sum on host.
"""
import os
import sys

for _p in ("/opt/trn_rl_repo", "/root/.axon_site/_ro/trn_rl_repo"):
    if os.path.isdir(_p) and _p not in sys.path:
        sys.path.insert(0, _p)

import numpy as np
import concourse.bass as bass
import concourse.mybir as mybir
import concourse.tile as tile
from concourse.bass_utils import run_bass_kernel_spmd
from concourse.masks import make_identity

# problem constants (hardcoded per spec)
N, E, G = 100000, 400000, 2000
D = 128
NC = 300           # RBF centers
CUT_LO, CUT_HI = 0.0, 30.0
N_CONV = 3
NCORES = 8
P = 128
NT = 99                      # node tiles per core
NTILES = NCORES * NT         # 792 global tiles
N_PAD = NT * P               # 12672 rows per core
DE = 428

F32 = mybir.dt.float32
F16 = mybir.dt.float16
F8 = mybir.dt.float8e4
I32 = mybir.dt.int32
CC_DT = F8            # dtype of the allgathered hn table (F8 = half AG wire)
AF = mybir.ActivationFunctionType
ALU = mybir.AluOpType

PAD_OFF = 200.0              # dst_off sentinel for padded edges
ABLATE = frozenset()         # timing ablations: nocc, nogather, nozchain, noconsume

# K-chunking of the 428-dim edge feature axis: emb 0:128 | rbf 128:428
KCH = [(0, 128), (128, 256), (256, 384), (384, 428)]
VCH = [(0, 128), (128, 256), (256, 300)]                 # rbf center chunks
ECH = [(0, 128), (128, 256), (256, 384), (384, 400)]     # edge-type chunks


def split_waits(nc):
    """Walrus allows only 1 sync wait per instruction; hoist extras onto
    preceding NoOps on the same engine."""
    n_fix = 0
    for f in nc.m.functions:
        for blk in f.blocks:
            out = []
            for inst in blk.instructions:
                si = inst.sync_info
                if si and len(si.on_wait) > 1 and not isinstance(inst, mybir.InstNoOp):
                    waits = list(si.on_wait)
                    for w in waits[:-1]:
                        nop = mybir.InstNoOp(name=f"{inst.name}-ws{n_fix}", ins=[], outs=[])
                        nop.engine = inst.engine
                        nop.sync_info = mybir.SyncInfo(on_wait=[w], on_update=[])
                        out.append(nop)
                        n_fix += 1
                    si.on_wait = [waits[-1]]
                out.append(inst)
            blk.instructions[:] = out
    return n_fix


def host_prep(inputs):
    """Balance nodes into tiles, shard edges, build per-core arrays."""
    Z = np.asarray(inputs["Z"]).astype(np.int64)
    etype = np.asarray(inputs["edge_type"]).astype(np.int64)
    dist = np.asarray(inputs["dist"]).astype(np.float32)
    src = np.asarray(inputs["src"]).astype(np.int64)
    dst = np.asarray(inputs["dst"]).astype(np.int64)
    gids = np.asarray(inputs["graph_ids"]).astype(np.int64)

    # snake-pack nodes (sorted by in-degree desc) into NTILES tiles
    deg = np.bincount(dst, minlength=N)
    order = np.argsort(-deg, kind="stable")
    j = np.arange(N)
    pos = j % (2 * NTILES)
    tile_seq = np.where(pos < NTILES, pos, 2 * NTILES - 1 - pos)
    node_tile = np.empty(N, np.int64)
    node_tile[order] = tile_seq
    ncnt = np.bincount(node_tile, minlength=NTILES)
    assert ncnt.max() <= P, f"tile node overflow {ncnt.max()}"
    ecnt = np.bincount(node_tile[dst], minlength=NTILES)
    # refinement: move low-degree nodes out of tiles that exceed 4*P edges
    cap = 4 * P
    for _ in range(64):
        t = int(np.argmax(ecnt))
        if ecnt[t] <= cap:
            break
        need = ecnt[t] - cap
        cand = np.where((node_tile == t) & (deg >= need))[0]
        mv = cand[np.argmin(deg[cand])]
        tgt = int(np.argmin(ecnt + (ncnt >= P) * E))
        node_tile[mv] = tgt
        ecnt[t] -= deg[mv]
        ecnt[tgt] += deg[mv]
        ncnt[t] -= 1
        ncnt[tgt] += 1
    tmax = max(1, int(np.ceil(ecnt.max() / P)))

    nstarts = np.zeros(NTILES + 1, np.int64)
    np.cumsum(ncnt, out=nstarts[1:])
    idx_sorted = np.lexsort((j, node_tile))
    slot = np.empty(N, np.int64)
    slot[idx_sorted] = j - nstarts[node_tile[idx_sorted]]
    node_row = node_tile * P + slot          # global padded row, < NCORES*N_PAD

    # edges sorted by dst tile
    etile = node_tile[dst]
    eorder = np.argsort(etile, kind="stable")
    etile_s = etile[eorder]
    estarts = np.zeros(NTILES + 1, np.int64)
    np.cumsum(ecnt, out=estarts[1:])
    epos = np.arange(E) - estarts[etile_s]
    lt = etile_s % NT
    ecore = etile_s // NT
    n_sub = NT * tmax
    e_cols = n_sub * P
    eslot = lt * (tmax * P) + epos
    src_row_s = node_row[src[eorder]].astype(np.int32)
    doff_s = slot[dst[eorder]].astype(np.float32)
    dist_s = dist[eorder]
    et_s = etype[eorder].astype(np.float32)

    def to_pf(arr):
        return np.ascontiguousarray(arr.reshape(n_sub, P).T)

    core_in = []
    for c in range(NCORES):
        m = ecore == c
        sl = eslot[m]
        sr = np.zeros(e_cols, np.int32)
        doffv = np.full(e_cols, PAD_OFF, dtype=np.float32)
        dd = np.zeros(e_cols, np.float32)
        et = np.zeros(e_cols, np.float32)
        sr[sl] = src_row_s[m]
        doffv[sl] = doff_s[m]
        dd[sl] = dist_s[m]
        et[sl] = et_s[m]
        x3 = np.stack([dd, dd * dd, np.ones_like(dd)], 0).astype(np.float32)
        zr = np.zeros(N_PAD, np.float16)
        nm = (node_tile // NT) == c
        zr[(node_tile[nm] % NT) * P + slot[nm]] = Z[nm].astype(np.float16)
        core_in.append(dict(
            src_row=to_pf(sr), dst_off=to_pf(doffv), x3=x3,
            ety_row=et.astype(np.float16).reshape(1, e_cols),
            z_row=zr.reshape(1, N_PAD),
        ))

    w = {}
    centers = np.linspace(CUT_LO, CUT_HI, NC, dtype=np.float32)
    gap = np.float32(centers[1] - centers[0])
    w["A"] = np.stack([2.0 * centers / gap,
                       -np.ones(NC, np.float32) / gap,
                       -(centers ** 2) / gap], 0).astype(np.float32)   # [3, NC]
    w["node_emb"] = np.asarray(inputs["node_emb"]).astype(np.float16)
    eemb = np.asarray(inputs["edge_emb"]).astype(np.float16)
    for k, (t0, t1) in enumerate(ECH):
        w[f"eemb_{k}"] = np.ascontiguousarray(eemb[t0:t1, :])
    def to_f8(a):
        import ml_dtypes
        return a.astype(ml_dtypes.float8_e4m3fn)

    for i in range(N_CONV):
        w[f"wn1t_{i}"] = np.ascontiguousarray(np.asarray(inputs["Wn1"][i]).T).astype(np.float16)
        w[f"wn2t_{i}"] = np.ascontiguousarray(np.asarray(inputs["Wn2"][i]).T).astype(np.float16)
        we1t = np.zeros((512, DE), np.float32)
        we1t[:DE] = np.asarray(inputs["We1"][i]).T   # [DE, DE] zero-pad K to 512
        we2t = np.zeros((512, D), np.float32)
        we2t[:DE] = np.asarray(inputs["We2"][i]).T   # [DE, D]
        # DoubleRow packing: chunk c covers K rows [256c, 256c+256) as
        # [Ki=128, Ko=2, m]; pad rows (428:512) are zero. m padded to 448
        # so the Ko-dim step is a multiple of 16 (s3_lw dual-fp8 rule).
        for c in range(2):
            blk1 = np.zeros((P, 2, 448), np.float32)
            blk1[:, :, :DE] = we1t[256 * c:256 * (c + 1)].reshape(2, P, DE).transpose(1, 0, 2)
            w[f"we1dr_{i}_{c}"] = to_f8(np.ascontiguousarray(blk1))
            blk2 = we2t[256 * c:256 * (c + 1)].reshape(2, P, D).transpose(1, 0, 2)
            w[f"we2dr_{i}_{c}"] = to_f8(np.ascontiguousarray(blk2))
        w[f"wct_{i}"] = np.ascontiguousarray(np.asarray(inputs["Wc"][i]).T).astype(np.float16)
        w[f"bn1_{i}"] = np.asarray(inputs["bn1"][i]).reshape(D, 1).astype(np.float32)
        w[f"bn2_{i}"] = np.asarray(inputs["bn2"][i]).reshape(D, 1).astype(np.float32)
        be1 = np.zeros((P, 4), np.float32)
        for k, (k0, k1) in enumerate(KCH):
            be1[:k1 - k0, k] = np.asarray(inputs["be1"][i])[k0:k1]
        w[f"be1_{i}"] = be1
        w[f"be2_{i}"] = np.asarray(inputs["be2"][i]).reshape(D, 1).astype(np.float32)
        w[f"bc_{i}"] = np.ascontiguousarray(
            np.tile(np.asarray(inputs["bc"][i]).reshape(1, D), (1, 4))).astype(np.float16)
    w["wr1t"] = np.ascontiguousarray(np.asarray(inputs["Wr1"]).T).astype(np.float16)
    w["wr2t"] = np.ascontiguousarray(np.asarray(inputs["Wr2"]).T).astype(np.float16)
    w["br1"] = np.asarray(inputs["br1"]).reshape(D, 1).astype(np.float32)
    w["br2"] = np.full((D, 1), np.asarray(inputs["br2"]).reshape(()), dtype=np.float32)

    meta = dict(tmax=tmax)
    return core_in, w, meta, (node_row, gids)


def build_nc(meta, reps=1):
    tmax = meta["tmax"]
    n_sub = NT * tmax
    e_cols = n_sub * P

    nc = bass.Bass(num_devices=NCORES)
    t_in = {}

    def inp(name, shp, dt=F32):
        t_in[name] = nc.dram_tensor(name, shp, dt, kind="ExternalInput")
        return t_in[name]

    src_row = inp("src_row", [P, n_sub], I32)
    dst_off = inp("dst_off", [P, n_sub], F32)
    x3 = inp("x3", [3, e_cols], F32)
    ety_row = inp("ety_row", [1, e_cols], F16)
    z_row = inp("z_row", [1, N_PAD], F16)
    inp("A", [3, NC], F32)
    inp("node_emb", [20, D], F16)
    for k, (t0, t1) in enumerate(ECH):
        inp(f"eemb_{k}", [t1 - t0, D], F16)
    for i in range(N_CONV):
        for nm, shp, dt in (("wn1t", [D, D], F16), ("wn2t", [D, D], F16),
                            ("wct", [D, D], F16), ("bn1", [D, 1], F32),
                            ("bn2", [D, 1], F32), ("be1", [P, 4], F32),
                            ("be2", [D, 1], F32), ("bc", [1, 4 * D], F16)):
            inp(f"{nm}_{i}", shp, dt)
        for c in range(2):
            inp(f"we1dr_{i}_{c}", [P, 2, 448], F8)
            inp(f"we2dr_{i}_{c}", [P, 2, D], F8)
    inp("wr1t", [D, D], F16)
    inp("wr2t", [D, 1], F16)
    inp("br1", [D, 1], F32)
    inp("br2", [D, 1], F32)
    r_out = nc.dram_tensor("r_out", [N_PAD, 1], F32, kind="ExternalOutput")

    evt = nc.dram_tensor("evt", [P, 4 * e_cols], F8, kind="Internal")
    cc_in = [nc.dram_tensor(f"cc_in_{i}", [N_PAD, D], CC_DT, kind="Internal")
             for i in range(N_CONV)]
    cc_out = [nc.dram_tensor(f"cc_out_{i}", [NCORES * N_PAD, D], CC_DT,
                             kind="Internal", addr_space="Shared")
              for i in range(N_CONV)]

    with tile.TileContext(nc) as tc:
        with (
            tc.tile_pool(name="const", bufs=1) as cp,
            tc.tile_pool(name="sb", bufs=4) as sb,
            tc.tile_pool(name="misc", bufs=2) as mp,
            tc.tile_pool(name="ev", bufs=3) as ev,
            tc.tile_pool(name="zp", bufs=3) as zp,
            tc.tile_pool(name="gat", bufs=16) as gp,
            tc.tile_pool(name="pz1", bufs=2, space="PSUM") as pz1,
            tc.tile_pool(name="pz2", bufs=1, space="PSUM") as pz2,
            tc.tile_pool(name="ptw", bufs=1, space="PSUM") as ptw,
            tc.tile_pool(name="pm", bufs=1, space="PSUM") as pm,
            tc.tile_pool(name="pd", bufs=2, space="PSUM") as pd,
            tc.tile_pool(name="ptp", bufs=1, space="PSUM") as ptp,
        ):
            # ---------------- constants ----------------
            ident = cp.tile([P, P], F32)
            make_identity(nc, ident[:])
            ident16 = cp.tile([P, P], F16)
            nc.vector.tensor_copy(out=ident16[:], in_=ident[:])
            if CC_DT is F16:
                identcc = ident16
            else:
                identcc = cp.tile([P, P], CC_DT)
                nc.vector.tensor_copy(out=identcc[:], in_=ident[:])
            iota_i = cp.tile([P, P], I32)
            nc.gpsimd.iota(iota_i[:], pattern=[[1, P]], base=0, channel_multiplier=0)
            iota_f = cp.tile([P, P], F32)
            nc.vector.tensor_copy(out=iota_f[:], in_=iota_i[:])
            iotac_i = cp.tile([P, 1], I32)
            nc.gpsimd.iota(iotac_i[:], pattern=[[1, 1]], base=0, channel_multiplier=1)
            iotac_f = cp.tile([P, 1], F32)
            nc.vector.tensor_copy(out=iotac_f[:], in_=iotac_i[:])
            ones_row = cp.tile([1, P], F16)
            nc.vector.memset(ones_row[:], 1.0)

            def load_const(name, shp, dt=F32):
                tl = cp.tile(shp, dt, tag=name)
                nc.sync.dma_start(out=tl[:], in_=t_in[name][:, :])
                return tl

            A_sb = load_const("A", [3, NC])
            nemb = load_const("node_emb", [20, D], F16)
            eembs = [load_const(f"eemb_{k}", [t1 - t0, D], F16)
                     for k, (t0, t1) in enumerate(ECH)]
            wr1t_sb = load_const("wr1t", [D, D], F16)
            wr2t_sb = load_const("wr2t", [D, 1], F16)
            br1_sb = load_const("br1", [D, 1])
            br2_sb = load_const("br2", [D, 1])
            W = {}
            for i in range(N_CONV):
                for nm, shp, dt in (("wn1t", [D, D], F16), ("wn2t", [D, D], F16),
                                    ("wct", [D, D], F16), ("bn1", [D, 1], F32),
                                    ("bn2", [D, 1], F32), ("be1", [P, 4], F32),
                                    ("be2", [D, 1], F32), ("bc", [1, 4 * D], F16)):
                    W[f"{nm}_{i}"] = load_const(f"{nm}_{i}", shp, dt)
                for c in range(2):
                    for nm, wd in (("we1dr", 448), ("we2dr", D)):
                        tl = cp.tile([P, 2, wd], F8, tag=f"{nm}_{i}_{c}",
                                     name=f"{nm}_{i}_{c}")
                        nc.sync.dma_start(out=tl[:], in_=t_in[f"{nm}_{i}_{c}"][:, :, :])
                        W[f"{nm}_{i}_{c}"] = tl

            # persistent SBUF state
            h_fm = cp.tile([P, N_PAD], F16, tag="h_fm")       # h feature-major
            ee_sb = cp.tile([P, e_cols], F16, tag="ee_sb")    # per-layer ee
            doff_sb = cp.tile([P, n_sub], F32, tag="doff_sb")
            nc.sync.dma_start(out=doff_sb[:], in_=dst_off[:, :])
            sri = cp.tile([P, n_sub], I32, tag="sri_sb")
            nc.sync.dma_start(out=sri[:], in_=src_row[:, :])

            # pre-zero fp8 pool buffers whose ragged tails feed DoubleRow
            # matmuls (zero weights * NaN-pattern garbage would poison PSUM)
            for zi_ in range(3):
                et0 = ev.tile([P, 4 * 512], F8, tag="evt_t", name=f"et0_{zi_}")
                nc.vector.memset(et0[:], 0.0)
                for tg in ("z1r01", "z1r23"):
                    zr0 = zp.tile([P, 1024], F8, tag=tg, name=f"zr0_{tg}_{zi_}")
                    nc.vector.memset(zr0[:], 0.0)

            for rep_ in range(reps):
                # ---------------- h0 init: one-hot from node_emb ----------------
                for q0 in range(0, N_PAD, 512):
                    wdt = min(512, N_PAD - q0)
                    zq = mp.tile([1, 512], F16, tag="zq")
                    nc.sync.dma_start(out=zq[:, :wdt], in_=z_row[:, q0:q0 + wdt])
                    pb = pz1.tile([P, 512], F32, space="PSUM", tag="pz1")
                    nc.tensor.matmul(out=pb[:, :wdt], lhsT=ones_row[:],
                                     rhs=zq[:, :wdt], start=True, stop=True)
                    oh = mp.tile([P, 512], F16, tag="oh")
                    nc.vector.tensor_scalar(
                        out=oh[:20, :wdt], in0=pb[:20, :wdt], scalar1=iotac_f[:20, :1],
                        scalar2=0.0, op0=ALU.subtract, op1=ALU.is_equal)
                    ph = pz2.tile([P, 512], F32, space="PSUM", tag="pz2")
                    nc.tensor.matmul(out=ph[:, :wdt], lhsT=nemb[:],
                                     rhs=oh[:20, :wdt], start=True, stop=True)
                    nc.vector.tensor_copy(out=h_fm[:, q0:q0 + wdt], in_=ph[:, :wdt])

                # ---------------- layers ----------------
                def node_mlp_chunk(i, q0):
                    wdt = min(512, N_PAD - q0)
                    ps1 = pz1.tile([P, 512], F32, space="PSUM", tag="pz1")
                    nc.tensor.matmul(out=ps1[:, :wdt], lhsT=W[f"wn1t_{i}"][:],
                                     rhs=h_fm[:, q0:q0 + wdt], start=True, stop=True)
                    zb = sb.tile([P, 512], F16, tag="nmlp_z")
                    nc.scalar.activation(out=zb[:, :wdt], in_=ps1[:, :wdt],
                                         func=AF.Relu, bias=W[f"bn1_{i}"][:, :1])
                    ps2 = pz2.tile([P, 512], F32, space="PSUM", tag="pz2")
                    nc.tensor.matmul(out=ps2[:, :wdt], lhsT=W[f"wn2t_{i}"][:],
                                     rhs=zb[:, :wdt], start=True, stop=True)
                    hnb = sb.tile([P, 512], F16, tag="nmlp_hn")
                    nc.scalar.activation(out=hnb[:, :wdt], in_=ps2[:, :wdt],
                                         func=AF.Identity, bias=W[f"bn2_{i}"][:, :1])
                    for a in range(wdt // P):
                        pt = ptp.tile([P, P], F16, space="PSUM", tag="tp")
                        nc.tensor.transpose(out=pt[:], in_=hnb[:, a * P:(a + 1) * P],
                                            identity=ident16[:])
                        hnm = sb.tile([P, P], CC_DT, tag="hn_nm")
                        nc.vector.tensor_copy(out=hnm[:], in_=pt[:])
                        nc.sync.dma_start(
                            out=cc_in[i][q0 + a * P:q0 + (a + 1) * P, :], in_=hnm[:])

                def readout_chunk(q0):
                    wdt = min(512, N_PAD - q0)
                    ps1 = pz1.tile([P, 512], F32, space="PSUM", tag="pz1")
                    nc.tensor.matmul(out=ps1[:, :wdt], lhsT=wr1t_sb[:],
                                     rhs=h_fm[:, q0:q0 + wdt], start=True, stop=True)
                    qb = mp.tile([P, 512], F16, tag="qb")
                    nc.scalar.activation(out=qb[:, :wdt], in_=ps1[:, :wdt],
                                         func=AF.Relu, bias=br1_sb[:, :1])
                    for a in range(wdt // P):
                        prt = pm.tile([P, P], F32, space="PSUM", tag="pm")
                        nc.tensor.matmul(out=prt[:, :1], lhsT=qb[:, a * P:(a + 1) * P],
                                         rhs=wr2t_sb[:], start=True, stop=True)
                        rsb = mp.tile([P, 1], F32, tag="rsb")
                        nc.scalar.activation(out=rsb[:], in_=prt[:, :1], func=AF.Identity,
                                             bias=br2_sb[:, :1])
                        nc.sync.dma_start(out=r_out[q0 + a * P:q0 + (a + 1) * P, :],
                                          in_=rsb[:])

                def post_consume_chunk(i, q0):
                    # emitted interleaved into consume_i once h tiles are final
                    if i + 1 < N_CONV:
                        node_mlp_chunk(i + 1, q0)
                    else:
                        readout_chunk(q0)

                for i in range(N_CONV):
                    if i == 0:
                        for q0 in range(0, N_PAD, 512):
                            node_mlp_chunk(0, q0)

                    if "nocc" not in ABLATE:
                        nc.gpsimd.collective_compute(
                            "AllGather", ALU.bypass,
                            replica_groups=[list(range(NCORES))],
                            ins=[cc_in[i][:, :]], outs=[cc_out[i][:, :]])

                    # --- z-chain: build/load evt, compute ee into SBUF ---
                    for t in range(NT if "nozchain" not in ABLATE else 0):
                        ts_ = slice(t * 512, (t + 1) * 512)
                        evt_t = ev.tile([P, 4 * 512], F8, tag="evt_t")
                        if i == 0 and rep_ == 0 or "rebuild" in ABLATE:
                            x3t = mp.tile([3, 512], F32, tag="x3t")
                            nc.sync.dma_start(out=x3t[:], in_=x3[:, ts_])
                            ett = mp.tile([1, 512], F16, tag="ett")
                            nc.sync.dma_start(out=ett[:], in_=ety_row[:, ts_])
                            # edge-type one-hot -> emb plane
                            pb = pz1.tile([P, 512], F32, space="PSUM", tag="pz1")
                            nc.tensor.matmul(out=pb[:], lhsT=ones_row[:],
                                             rhs=ett[:], start=True, stop=True)
                            pacc = pz2.tile([P, 512], F32, space="PSUM", tag="pz2")
                            for k, (t0, t1) in enumerate(ECH):
                                oh = mp.tile([P, 512], F16, tag="oh")
                                nc.vector.tensor_scalar(
                                    out=oh[:t1 - t0, :], in0=pb[:t1 - t0, :],
                                    scalar1=iotac_f[:t1 - t0, :1], scalar2=float(t0),
                                    op0=ALU.subtract, op1=ALU.is_equal)
                                nc.tensor.matmul(out=pacc[:], lhsT=eembs[k][:],
                                                 rhs=oh[:t1 - t0, :],
                                                 start=(k == 0), stop=(k == 3))
                            nc.vector.tensor_copy(out=evt_t[:, 0:512], in_=pacc[:])
                            # rbf planes: V = exp(A @ x3)
                            for k, (c0, c1) in enumerate(VCH):
                                pv = pz1.tile([P, 512], F32, space="PSUM", tag="pz1")
                                nc.tensor.matmul(out=pv[:c1 - c0, :],
                                                 lhsT=A_sb[:, c0:c1],
                                                 rhs=x3t[:],
                                                 start=True, stop=True)
                                nc.scalar.activation(
                                    out=evt_t[:c1 - c0, (k + 1) * 512:(k + 1) * 512 + 512],
                                    in_=pv[:c1 - c0, :], func=AF.Exp)
                            nc.sync.dma_start(
                                out=evt[:, t * 2048:(t + 1) * 2048], in_=evt_t[:])
                        else:
                            nc.sync.dma_start(
                                out=evt_t[:], in_=evt[:, t * 2048:(t + 1) * 2048])
                        # z1 m-chunks via fp8 DoubleRow (K packed 2-per-cell)
                        zr01 = zp.tile([P, 1024], F8, tag="z1r01")
                        zr23 = zp.tile([P, 1024], F8, tag="z1r23")
                        zrs = [zr01, zr23]
                        for mi, (m0, m1) in enumerate(KCH):
                            pz = pz1.tile([P, 512], F32, space="PSUM", tag="pz1")
                            for c in range(2):
                                nc.tensor.matmul(
                                    out=pz[:m1 - m0, :],
                                    lhsT=W[f"we1dr_{i}_{c}"][:, :, m0:m1],
                                    rhs=evt_t[:, c * 1024:(c + 1) * 1024].rearrange(
                                        "p (k n) -> p k n", k=2),
                                    start=(c == 0), stop=(c == 1),
                                    perf_mode=mybir.MatmulPerfMode.DoubleRow)
                            zsl = zrs[mi // 2][:m1 - m0, (mi % 2) * 512:(mi % 2) * 512 + 512]
                            if mi < 2:
                                nc.scalar.activation(out=zsl, in_=pz[:m1 - m0, :],
                                                     func=AF.Relu,
                                                     bias=W[f"be1_{i}"][:m1 - m0, mi:mi + 1])
                            else:
                                nc.vector.tensor_scalar(
                                    out=zsl, in0=pz[:m1 - m0, :],
                                    scalar1=W[f"be1_{i}"][:m1 - m0, mi:mi + 1],
                                    scalar2=0.0, op0=ALU.add, op1=ALU.max)
                        # z2 -> ee (SBUF)
                        pe_ = pz2.tile([P, 512], F32, space="PSUM", tag="pz2")
                        for c in range(2):
                            nc.tensor.matmul(
                                out=pe_[:], lhsT=W[f"we2dr_{i}_{c}"][:, :, :],
                                rhs=zrs[c][:].rearrange("p (k n) -> p k n", k=2),
                                start=(c == 0), stop=(c == 1),
                                perf_mode=mybir.MatmulPerfMode.DoubleRow)
                        nc.scalar.activation(out=ee_sb[:, ts_], in_=pe_[:],
                                             func=AF.Identity, bias=W[f"be2_{i}"][:, :1])

                    # --- consume: gather hn, product, message, scatter ---
                    for t in range(NT if "noconsume" not in ABLATE else 0):
                        ts_ = slice(t * 512, (t + 1) * 512)
                        pw = ptw.tile([P, 512], F16, space="PSUM", tag="tpw")
                        for a in range(tmax):
                            s = t * tmax + a
                            g = gp.tile([P, D], CC_DT, tag="gath16")
                            if "nogather" in ABLATE:
                                nc.gpsimd.dma_start(out=g[:], in_=cc_out[i][0:P, :])
                            else:
                                nc.gpsimd.indirect_dma_start(
                                    out=g[:], out_offset=None, in_=cc_out[i][:, :],
                                    in_offset=bass.IndirectOffsetOnAxis(
                                        ap=sri[:, s:s + 1], axis=0))
                            if CC_DT is F16:
                                g16 = g
                            else:
                                g16 = gp.tile([P, D], F16, tag="g16")
                                nc.vector.tensor_copy(out=g16[:], in_=g[:])
                            nc.tensor.transpose(out=pw[:, a * P:(a + 1) * P], in_=g16[:],
                                                identity=ident16[:])
                        hnf = sb.tile([P, 512], F16, tag="hnf")
                        nc.scalar.copy(out=hnf[:], in_=pw[:])
                        prod = sb.tile([P, 512], F16, tag="prod")
                        nc.vector.tensor_mul(out=prod[:], in0=hnf[:], in1=ee_sb[:, ts_])
                        pmw = pm.tile([P, 512], F32, space="PSUM", tag="pm")
                        nc.tensor.matmul(out=pmw[:], lhsT=ones_row[:],
                                         rhs=W[f"bc_{i}"][:], start=True, stop=False)
                        for a in range(tmax):
                            nc.tensor.matmul(out=pmw[:, a * P:(a + 1) * P],
                                             lhsT=prod[:, a * P:(a + 1) * P],
                                             rhs=W[f"wct_{i}"][:], start=False,
                                             stop=True, skip_group_check=True)
                        msb = sb.tile([P, 512], F16, tag="msb")
                        nc.scalar.activation(out=msb[:], in_=pmw[:], func=AF.Tanh)
                        # scatter: out[dim, node] = sum_e msb[e, dim] * S[e, node]
                        pdt = pd.tile([P, P], F32, space="PSUM", tag="pd")
                        for a in range(tmax):
                            s = t * tmax + a
                            S = sb.tile([P, P], F16, tag="S")
                            nc.vector.tensor_tensor(
                                out=S[:], in0=doff_sb[:, s:s + 1].to_broadcast([P, P]),
                                in1=iota_f[:], op=ALU.is_equal)
                            nc.tensor.matmul(out=pdt[:], lhsT=msb[:, a * P:(a + 1) * P],
                                             rhs=S[:], start=(a == 0), stop=(a == tmax - 1))
                        nc.vector.tensor_add(
                            out=h_fm[:, t * P:(t + 1) * P],
                            in0=h_fm[:, t * P:(t + 1) * P], in1=pdt[:])
                        if (t + 1) % 4 == 0 or t == NT - 1:
                            post_consume_chunk(i, (t // 4) * 512)

                    if "noconsume" in ABLATE:
                        # interleaved chunks were skipped; emit plainly
                        for q0 in range(0, N_PAD, 512):
                            post_consume_chunk(i, q0)
    return nc


_CACHE = {}


def _get_runner(meta, reps=1):
    key = (tuple(sorted(meta.items())), reps, ABLATE)
    if key not in _CACHE:
        nc = build_nc(meta, reps=reps)
        nc.finalize()
        split_waits(nc)
        _CACHE[key] = nc
    return _CACHE[key]


def kernel(**inputs):
    core_in, w, meta, (node_row, gids) = host_prep(inputs)
    nc = _get_runner(meta)
    in_maps = []
    for c in range(NCORES):
        m = dict(core_in[c])
        m.update(w)
        in_maps.append(m)
    res = run_bass_kernel_spmd(nc, in_maps, core_ids=list(range(NCORES)))
    r_all = np.concatenate([res.results[c]["r_out"][:, 0] for c in range(NCORES)])
    r_full = r_all[node_row]
    out = np.bincount(gids, weights=r_full.astype(np.float64), minlength=G)[:G]
    return out.astype(np.float32)
